# revision 1
# baseline (speedup 1.0000x reference)
"""Trainium2 Bass kernel for nn_DiffeqExactTraceMLP.

Math (B=1024, D=128, DH=64, H=512):
  h = MADE_fwd(x) + MADE_rev(x)                       # hollow conditioner
  u[b,i] = [t, x[b,i], h[b,i,:]]                      # [B, D, DH+2]
  y   = MLP(u)        (tanh, tanh, linear->scalar)    # per-dim MLP
  jac = exact JVP of MLP wrt the x slot of u

Sharding: pure data parallelism over batch across 8 cores (128 rows each).
Weights replicated; all masking / transposes / t-folding done on host.

v4: ALL device inputs packed into a single bf16 DRAM tensor wA (f32 bias
columns carried as bf16 hi+lo pairs, recombined on device with one DVE
add); single packed [2, ROWS] f32 output. Per-dispatch buffer count 3
(each extra sharded buffer costs ~10us of dispatch overhead).

Per-core device schedule:
  MADE runs in transposed-activation space (features on partitions),
  its last layer lands batch-major ([128 batch, 8192]) so a 64-way DMA
  scatter builds UT = [x_flat; h feats] with rows r = b*128 + i.
  Dimwise MLP runs feature-on-partition over row blocks of 1024:
    z1T  = tanh(W0x^T UT + bias0_eff)     bias0_eff = d_b0 + t*d_W0[0]
    z1dT = a - a*z1T^2                    a = d_W0[1]
    p2  = W1^T z1T   (PSUM, fp32)
    p2n = W1n^T z1dT with W1n = d_W1 * (-w2) folded on host
    z2  = tanh(p2 + d_b1)
    Sy += w2[m] * z2          (DVE accumulate over the 4 feature chunks)
    Sj += (z2^2 - 1) * p2n    ( = w2 * z2d, sign folded into W1n )
    y/jac = ones^T Sy / ones^T Sj   (K=128 reduction matmuls)
Matmuls in bf16 (fp32 PSUM accumulation).
"""

import numpy as np
import ml_dtypes

B, D, DH, H = 1024, 128, 64, 512
NCORES = 8
BS = B // NCORES          # batch rows per core = 128
ROWS = BS * D             # dimwise rows per core = 16384
RBLK = 1024               # dimwise row-block
NBLK = ROWS // RBLK       # 16
KIN = DH + 1              # 65 = [x, h0..h63]

_BF = ml_dtypes.bfloat16

# wA [512, A_COLS] bf16 column offsets. Rows 0:512 for the 512-row weights;
# the misc region (cols >= 18432) only uses rows 0:128 (rest zeros).
A_W1M1, A_W1M2, A_DW1, A_DW1N, A_W2M1, A_W2M2 = 0, 512, 1024, 1536, 2048, 10240
M_X, M_XT, M_W0M1, M_W0M2, M_DW0X = 18432, 18560, 18688, 19200, 19712
M_B2S, M_WDHI, M_WDLO = 20224, 20288, 20324
A_COLS = 20360
# wd (f32, [128, 36]) column offsets (chunk-col layout: v[c*128+p] at [p, c])
D_B0C = {"m1": 0, "m2": 4}
D_B1C = {"m1": 8, "m2": 12}
D_BIAS0, D_B1D, D_AC, D_NEGAC, D_W2 = 16, 20, 24, 28, 32
D_COLS = 36


def _bf(x):
    return np.ascontiguousarray(np.asarray(x, dtype=np.float32).astype(_BF))


def _chunk_col(v):
    """[512] -> [128, 4] with v[c*128 + p] at [p, c] (per-partition scalars)."""
    return np.ascontiguousarray(np.asarray(v, np.float32).reshape(4, 128).T)


def _made_masks(reverse):
    deg_in = np.arange(D)
    if reverse:
        deg_in = deg_in[::-1].copy()
    deg_h = np.arange(H) % (D - 1)
    degs = [deg_in, deg_h, deg_h]
    masks = [(d0[:, None] <= d1[None, :]) for d0, d1 in zip(degs[:-1], degs[1:])]
    out = degs[-1][:, None] < deg_in[None, :]
    masks.append(np.tile(out, (1, DH)))
    return [m.astype(np.float32) for m in masks]


_NC_CACHE = {}


def _build_nc():
    import os
    unroll = int(os.environ.get("BENCH_UNROLL", "1"))
    if unroll in _NC_CACHE:
        return _NC_CACHE[unroll]
    import concourse.bacc as bacc
    import concourse.mybir as mybir
    from concourse.tile import TileContext

    dt = mybir.dt
    AF = mybir.ActivationFunctionType
    OP = mybir.AluOpType

    nc = bacc.Bacc(None, target_bir_lowering=False)

    wA = nc.declare_dram_parameter("wA", [512, A_COLS], dt.bfloat16, isOutput=False)
    yj = nc.declare_dram_parameter("yj", [2, ROWS], dt.float32, isOutput=True)

    with TileContext(nc) as tc:
        _pools = []

        def _pool(**kw):
            p = tc.alloc_tile_pool(**kw)
            _pools.append(p)
            return p

        cpool = _pool(name="const", bufs=1)
        hpool = _pool(name="made_h", bufs=16)
        wspool = _pool(name="w2s", bufs=6)
        zpool = _pool(name="z", bufs=8)
        z1dpool = _pool(name="z1d", bufs=8)
        z2pool = _pool(name="z2", bufs=4)
        tpool = _pool(name="tmp", bufs=3)
        sqpool = _pool(name="sq", bufs=3)
        spool = _pool(name="S", bufs=4)
        outpool = _pool(name="outp", bufs=4)

        _dma_engines = [nc.sync, nc.scalar]
        _dma_i = [0]

        def dma_rr(out, in_):
            eng = _dma_engines[_dma_i[0] % len(_dma_engines)]
            _dma_i[0] += 1
            eng.dma_start(out=out, in_=in_)

        def ctile(shape, dtype, name):
            return cpool.tile(list(shape), dtype, tag=name, name=name)

        xT_sb = ctile((128, BS), dt.bfloat16, "xT_sb")
        nc.sync.dma_start(out=xT_sb[:], in_=wA[0:128, M_XT:M_XT + BS])
        w0m_sb = {}
        for p, off in (("m1", M_W0M1), ("m2", M_W0M2)):
            t = ctile((128, H), dt.bfloat16, f"w0m_{p}")
            nc.sync.dma_start(out=t[:], in_=wA[0:128, off:off + H])
            w0m_sb[p] = t
        dw0x_sb = ctile((KIN, H), dt.bfloat16, "dw0x_sb")
        nc.sync.dma_start(out=dw0x_sb[:], in_=wA[0:KIN, M_DW0X:M_DW0X + H])
        b2sum_sb = ctile((1, D * DH), dt.bfloat16, "b2sum_sb")
        nc.sync.dma_start(out=b2sum_sb[:], in_=wA[0:128, M_B2S:M_B2S + 64])
        wd_hi = ctile((128, D_COLS), dt.bfloat16, "wd_hi")
        nc.sync.dma_start(out=wd_hi[:], in_=wA[0:128, M_WDHI:M_WDHI + D_COLS])
        wd_lo = ctile((128, D_COLS), dt.bfloat16, "wd_lo")
        nc.sync.dma_start(out=wd_lo[:], in_=wA[0:128, M_WDLO:M_WDLO + D_COLS])
        wd_sb = ctile((128, D_COLS), dt.float32, "wd_sb")
        nc.vector.tensor_tensor(wd_sb[:], wd_hi[:], wd_lo[:], op=OP.add)

        # [512, 512] weights land as [128, 4*512] with row-chunk k at cols k*512
        def load_512(tag, col0):
            t = ctile((128, 4 * H), dt.bfloat16, tag)
            for k in range(4):
                nc.sync.dma_start(
                    out=t[:, k * H:(k + 1) * H],
                    in_=wA[k * 128:(k + 1) * 128, col0:col0 + H],
                )
            return t

        w1m_sb = {"m1": load_512("w1m_m1", A_W1M1), "m2": load_512("w1m_m2", A_W1M2)}
        dw1_sb = load_512("dw1_sb", A_DW1)
        dw1n_sb = load_512("dw1n_sb", A_DW1N)

        ones_row = ctile((1, 128), dt.bfloat16, "ones_row")
        nc.vector.memset(ones_row[:], 1.0)
        ones_col = ctile((128, 1), dt.bfloat16, "ones_col")
        nc.vector.memset(ones_col[:], 1.0)

        def wdc(col):
            return wd_sb[:, col:col + 1]

        # repeated body for benchmarking (BENCH_UNROLL>1); rep results identical
        for _rep in range(unroll):
            made_sb = ctile((BS, D * DH), dt.bfloat16, "made_sb")
            UT = ctile((KIN, ROWS), dt.bfloat16, "UT")

            # ---------------- MADE (both orderings) ----------------
            madeps = tc.alloc_tile_pool(name="madeps", bufs=8, space="PSUM")
            h1T = {}
            h2T = {}
            for p in ("m1", "m2"):
                for c in range(4):
                    ps = madeps.tile([128, BS], dt.float32, tag="mps", name="mps")
                    nc.tensor.matmul(
                        ps[:], w0m_sb[p][:, c * 128:(c + 1) * 128], xT_sb[:],
                        start=True, stop=True,
                    )
                    h = hpool.tile([128, BS], dt.bfloat16, tag="h", name="h")
                    nc.scalar.activation(h[:], ps[:], AF.Relu, bias=wdc(D_B0C[p] + c))
                    h1T[p, c] = h
            for p in ("m1", "m2"):
                for m in range(4):
                    ps = madeps.tile([128, BS], dt.float32, tag="mps", name="mps")
                    for k in range(4):
                        nc.tensor.matmul(
                            ps[:],
                            w1m_sb[p][:, k * H + m * 128: k * H + (m + 1) * 128],
                            h1T[p, k][:],
                            start=(k == 0), stop=(k == 3),
                        )
                    h = hpool.tile([128, BS], dt.bfloat16, tag="h2", name="h2")
                    nc.scalar.activation(h[:], ps[:], AF.Relu, bias=wdc(D_B1C[p] + m))
                    h2T[p, m] = h

            # MADE layer 2, batch-major PSUM [BS, 512] per n-block; groups of <=6
            groups = [list(range(0, 6)), list(range(6, 12)), list(range(12, 16))]
            for grp in groups:
                pstiles = {n: madeps.tile([BS, 512], dt.float32, tag="mps", name="mps")
                           for n in grp}
                first = True
                for p, coff in (("m1", A_W2M1), ("m2", A_W2M2)):
                    for k in range(4):
                        for ni in range(0, len(grp), 2):
                            n0 = grp[ni]
                            w2t = wspool.tile([128, 1024], dt.bfloat16, tag="w2t", name="w2t")
                            dma_rr(
                                w2t[:],
                                wA[k * 128:(k + 1) * 128,
                                   coff + n0 * 512: coff + (n0 + 2) * 512],
                            )
                            for d in range(2):
                                nc.tensor.matmul(
                                    pstiles[n0 + d][:], h2T[p, k][:],
                                    w2t[:, d * 512:(d + 1) * 512],
                                    start=first, stop=False,
                                )
                        first = False
                for n in grp:
                    nc.tensor.matmul(
                        pstiles[n][:], ones_row[:],
                        b2sum_sb[:, n * 512:(n + 1) * 512],
                        start=False, stop=True,
                    )
                    if n % 2 == 0:
                        nc.scalar.activation(
                            made_sb[:, n * 512:(n + 1) * 512], pstiles[n][:], AF.Copy
                        )
                    else:
                        nc.vector.tensor_copy(
                            made_sb[:, n * 512:(n + 1) * 512], pstiles[n][:]
                        )

            # ---------------- UT assembly ----------------
            dma_rr(UT[0:1, :], wA[0:128, M_X:M_X + 128])
            for k in range(DH):
                dma_rr(UT[1 + k:2 + k, :], made_sb[:, k * 128:(k + 1) * 128])

            madeps.release()
            pspool = tc.alloc_tile_pool(name="ps", bufs=3, space="PSUM")
            psf = tc.alloc_tile_pool(name="psf", bufs=2, space="PSUM")

            # ---------------- dimwise MLP over row blocks ----------------
            # L0 for block b+1 is emitted between L1(b) and final(b): PE fills
            # the stall where it would wait on ACT/DVE producing z2/Sy/Sj(b),
            # and z1(b+1) is ready before L1(b+1) begins.
            def do_L0(b):
                base = b * RBLK
                z1 = {}
                z1d = {}
                for m in range(4):
                    zt = zpool.tile([128, RBLK], dt.bfloat16, tag="z1", name="z1")
                    ps = pspool.tile([128, RBLK], dt.float32, tag="ps", name="psL0")
                    for s in range(2):
                        nc.tensor.matmul(
                            ps[:, s * 512:(s + 1) * 512],
                            dw0x_sb[:, m * 128:(m + 1) * 128],
                            UT[:, base + s * 512: base + (s + 1) * 512],
                            start=True, stop=True,
                        )
                    nc.scalar.activation(zt[:], ps[:], AF.Tanh, bias=wdc(D_BIAS0 + m))
                    sq = sqpool.tile([128, RBLK], dt.bfloat16, tag="sq1", name="sq1")
                    nc.vector.tensor_tensor(sq[:], zt[:], zt[:], op=OP.mult)
                    zd = z1dpool.tile([128, RBLK], dt.bfloat16, tag="z1d", name="z1d")
                    nc.vector.tensor_scalar(
                        zd[:], sq[:], wdc(D_NEGAC + m), wdc(D_AC + m),
                        op0=OP.mult, op1=OP.add,
                    )
                    z1[m] = zt
                    z1d[m] = zd
                return z1, z1d

            cur = do_L0(0)
            for b in range(NBLK):
                base = b * RBLK
                z1, z1d = cur
                Sy = spool.tile([128, RBLK], dt.bfloat16, tag="Sy", name="Sy")
                Sj = spool.tile([128, RBLK], dt.bfloat16, tag="Sj", name="Sj")
                for m in range(4):
                    p2 = pspool.tile([128, RBLK], dt.float32, tag="ps", name="p2ps")
                    p2n = pspool.tile([128, RBLK], dt.float32, tag="ps", name="p2nps")
                    for k in range(4):
                        lhs = dw1_sb[:, k * H + m * 128: k * H + (m + 1) * 128]
                        lhsn = dw1n_sb[:, k * H + m * 128: k * H + (m + 1) * 128]
                        for s in range(2):
                            sl = slice(s * 512, (s + 1) * 512)
                            nc.tensor.matmul(
                                p2[:, sl], lhs, z1[k][:, sl],
                                start=(k == 0), stop=(k == 3),
                            )
                            nc.tensor.matmul(
                                p2n[:, sl], lhsn, z1d[k][:, sl],
                                start=(k == 0), stop=(k == 3),
                            )
                    z2t = z2pool.tile([128, RBLK], dt.bfloat16, tag="z2", name="z2")
                    nc.scalar.activation(z2t[:], p2[:], AF.Tanh, bias=wdc(D_B1D + m))
                    sq = sqpool.tile([128, RBLK], dt.bfloat16, tag="sq2", name="sq2")
                    nc.vector.tensor_tensor(sq[:], z2t[:], z2t[:], op=OP.mult)
                    # Sj += (z2^2 - 1) * p2n  ( = w2*z2d; -w2 folded into W1n )
                    if m == 0:
                        nc.vector.scalar_tensor_tensor(
                            Sj[:], sq[:], 1.0, p2n[:], op0=OP.subtract, op1=OP.mult,
                        )
                        nc.vector.tensor_scalar(
                            Sy[:], z2t[:], wdc(D_W2 + 0), None, op0=OP.mult,
                        )
                    else:
                        zdt = tpool.tile([128, RBLK], dt.bfloat16, tag="zdt", name="zdt")
                        nc.vector.scalar_tensor_tensor(
                            zdt[:], sq[:], 1.0, p2n[:], op0=OP.subtract, op1=OP.mult,
                        )
                        nc.vector.tensor_tensor(Sj[:], zdt[:], Sj[:], op=OP.add)
                        nc.vector.scalar_tensor_tensor(
                            Sy[:], z2t[:], wdc(D_W2 + m), Sy[:],
                            op0=OP.mult, op1=OP.add,
                        )
                if b + 1 < NBLK:
                    cur = do_L0(b + 1)
                for s in range(2):
                    sl = slice(s * 512, (s + 1) * 512)
                    py = psf.tile([1, 512], dt.float32, tag="psf", name="pyf")
                    nc.tensor.matmul(py[:], ones_col[:], Sy[:, sl], start=True, stop=True)
                    ysb = outpool.tile([1, 512], dt.float32, tag="ysb", name="ysb")
                    nc.scalar.activation(ysb[:], py[:], AF.Copy)
                    nc.sync.dma_start(
                        out=yj[0:1, base + s * 512: base + (s + 1) * 512], in_=ysb[:],
                    )
                    pj = psf.tile([1, 512], dt.float32, tag="psf", name="pjf")
                    nc.tensor.matmul(pj[:], ones_col[:], Sj[:, sl], start=True, stop=True)
                    jsb = outpool.tile([1, 512], dt.float32, tag="jsb", name="jsb")
                    nc.vector.tensor_copy(jsb[:], pj[:])
                    nc.sync.dma_start(
                        out=yj[1:2, base + s * 512: base + (s + 1) * 512], in_=jsb[:],
                    )

            psf.release()
            pspool.release()
        for p in reversed(_pools):
            p.release()

    nc.compile()
    _NC_CACHE[unroll] = nc
    return nc


def _host_prep(inputs):
    """Build the per-core input maps (numpy only)."""
    t = np.asarray(inputs["t"], np.float32)
    x = np.asarray(inputs["x"], np.float32)
    M = {"m1": _made_masks(False), "m2": _made_masks(True)}

    d_W0 = np.asarray(inputs["d_W0"], np.float32)
    d_b0 = np.asarray(inputs["d_b0"], np.float32)
    d_W1 = np.asarray(inputs["d_W1"], np.float32)
    w2 = np.asarray(inputs["d_W2"], np.float32)[:, 0]

    wAf = np.zeros((512, A_COLS), np.float32)
    wAf[:, A_W1M1:A_W1M1 + H] = np.asarray(inputs["m1_W1"], np.float32) * M["m1"][1]
    wAf[:, A_W1M2:A_W1M2 + H] = np.asarray(inputs["m2_W1"], np.float32) * M["m2"][1]
    wAf[:, A_DW1:A_DW1 + H] = d_W1
    wAf[:, A_DW1N:A_DW1N + H] = d_W1 * (-w2)[None, :]
    wAf[:, A_W2M1:A_W2M1 + D * DH] = np.asarray(inputs["m1_W2"], np.float32) * M["m1"][2]
    wAf[:, A_W2M2:A_W2M2 + D * DH] = np.asarray(inputs["m2_W2"], np.float32) * M["m2"][2]

    wAf[0:128, M_W0M1:M_W0M1 + H] = np.asarray(inputs["m1_W0"], np.float32) * M["m1"][0]
    wAf[0:128, M_W0M2:M_W0M2 + H] = np.asarray(inputs["m2_W0"], np.float32) * M["m2"][0]
    wAf[0:KIN, M_DW0X:M_DW0X + H] = d_W0[1:, :]
    b2s = np.asarray(inputs["m1_b2"], np.float32) + np.asarray(inputs["m2_b2"], np.float32)
    wAf[0:128, M_B2S:M_B2S + 64] = b2s.reshape(128, 64)

    wDm = np.zeros((128, D_COLS), np.float32)
    for p in ("m1", "m2"):
        wDm[:, D_B0C[p]:D_B0C[p] + 4] = _chunk_col(inputs[f"{p}_b0"])
        wDm[:, D_B1C[p]:D_B1C[p] + 4] = _chunk_col(inputs[f"{p}_b1"])
    wDm[:, D_BIAS0:D_BIAS0 + 4] = _chunk_col(d_b0 + t[0] * d_W0[0, :])
    wDm[:, D_B1D:D_B1D + 4] = _chunk_col(inputs["d_b1"])
    a = d_W0[1, :]
    wDm[:, D_AC:D_AC + 4] = _chunk_col(a)
    wDm[:, D_NEGAC:D_NEGAC + 4] = _chunk_col(-a)
    wDm[:, D_W2:D_W2 + 4] = _chunk_col(w2)
    wd_hi = wDm.astype(_BF)
    wd_lo = (wDm - wd_hi.astype(np.float32)).astype(_BF)
    wAf[0:128, M_WDHI:M_WDHI + D_COLS] = wd_hi.astype(np.float32)
    wAf[0:128, M_WDLO:M_WDLO + D_COLS] = wd_lo.astype(np.float32)

    wA_common = wAf.astype(_BF)

    in_maps = []
    for c in range(NCORES):
        xs = x[c * BS:(c + 1) * BS, :]
        wAc = wA_common.copy()
        wAc[0:128, M_X:M_X + 128] = xs.astype(_BF)
        wAc[0:128, M_XT:M_XT + BS] = xs.T.astype(_BF)
        in_maps.append({"wA": wAc})
    return in_maps


def kernel(**inputs):
    from concourse.bass_utils import run_bass_kernel_spmd

    nc = _build_nc()
    in_maps = _host_prep(inputs)
    res = run_bass_kernel_spmd(nc, in_maps, list(range(NCORES)))

    d_b2 = np.asarray(inputs["d_b2"], np.float32)
    y = np.concatenate(
        [res.results[c]["yj"][0].reshape(BS, D) for c in range(NCORES)], axis=0
    ) + d_b2[0]
    jac = np.concatenate(
        [res.results[c]["yj"][1].reshape(BS, D) for c in range(NCORES)], axis=0
    )
    return np.asarray(y, np.float32), np.asarray(jac, np.float32)

